# revision 26
# baseline (speedup 1.0000x reference)
"""CostVolume kernel for Trainium2 (8 NeuronCores, Bass/Tile).

Math: the reference computes a 9x9-displacement correlation cost volume and
scatters it into out[b, r', c', r, c].  Substituting r' = r + di - 4,
c' = c + dj - 4 shows the output is a banded Gram matrix:

    out[b, r', c', r, c] = (sum_ch feat2[b,ch,r',c'] * feat1[b,ch,r,c])
                           * 1[|r'-r| <= 4] * 1[|c'-c| <= 4]

98% of the dense (B,H,W,H,W) output is structural zeros.  The device
computes and writes ONLY a compact band; the host applies the band mask
and scatters into a zero-initialized full-shape array during unsharding.

Sharding: 8 cores = 4 batches x 2 r'-halves (r' in [0,32) / [32,64)).

v2 layout (vs the 33us v1): chunks are 16x8 pixel tiles instead of 2x64.
A 128-partition chunk of (16 r' x 8 c') output pixels needs only the
f1 window (16+8) rows x (8+8) cols = 24x16 = 384 band columns, vs 640
for the old (2 r' x 64 c') shape.  384 f32 fits in ONE 2KiB PSUM bank,
so each (chunk, ch-half) is a single LDWEIGHTS+MATMUL pair: 32 pairs,
12288 streamed PE columns (vs 64 pairs / 20480 columns).  Cast work and
output bytes drop 1.67x as well (2.62 -> 1.57 MiB per core).

Per chunk q = 2*cc + rc (cc in 0..7 c'-blocks, rc in 0..1 r'-blocks):
  psum[p, cwin*24 + rwin] = sum_ch f2[ch, p] * f1[ch, 8cc+cwin, 16rc+rwin]
  with p = r_local*8 + c_local, f1 host-padded by 4 on all spatial edges.
bf16 matmul accumulated in fp32 PSUM, band cast to f16 on device, mask +
scatter on host.

Schedule (inherited from v1's perfetto/NTFF tuning):
- inputs stream across BOTH HWDGE queues (sync + scalar) in chunk-
  consumption order; a small h0-only head piece ("hd") lets the first
  matmul start ~1us before the first full f1/f2 pieces land.
- Vector casts groups 0-6, Scalar casts the last two groups (nothing
  follows them in its stream); ALL other out-DMA issues ride Sync, with
  two mid-stream writes on the otherwise-idle gpsimd SWDGE queue.
- group 0 and the last group are single-chunk: the first band write hits
  the output queue early, and the final cast+write tail is short.
"""

import numpy as np

B, C, H, W = 4, 256, 64, 64
MD = 4
N_CORES = 8
RSH = H // 2          # 32 r' rows per core
RQ, CQ = 16, 8        # chunk = 16 r' x 8 c' pixels
NRC, NCC = RSH // RQ, W // CQ   # 2 r'-blocks x 8 c'-blocks = 16 chunks
NQ = NRC * NCC
RWIN = RQ + 2 * MD    # 24 band rows per chunk
CWIN = CQ + 2 * MD    # 16 band cols per chunk
NW = RWIN * CWIN      # 384 band columns per chunk
FW = RSH + 2 * MD     # 40 padded f1 rows
FC = W + 2 * MD       # 72 padded f1 cols

# (group start chunk, chunks in group): doubles early (drain amortizes),
# singles for the last four chunks — small casts alternating across
# Vector/Scalar and small writes across three queues compress the tail
# between the last matmul and the NEFF teardown barrier
GROUPS = [(2 * g, 2) for g in range(6)] + [(q, 1) for q in range(12, 16)]

_COMPILED = None      # compiled Bacc program cache across kernel() calls


def _build_program():
    import concourse.bacc as bacc
    import concourse.tile as tile
    from concourse import mybir

    f32 = mybir.dt.float32
    f16 = mybir.dt.float16
    bf16 = mybir.dt.bfloat16

    nc = bacc.Bacc("TRN2", target_bir_lowering=False, debug=False,
                   num_devices=N_CORES)

    f2d = nc.dram_tensor("f2", [128, NQ, 2, 128], bf16,
                         kind="ExternalInput").ap()
    # f1 ships WITHOUT the 4-col zero pads on each side (64 real cols);
    # the pads are memset on-device, saving 160KB/core of HBM traffic
    f1d = nc.dram_tensor("f1", [128, W, 2, FW], bf16,
                         kind="ExternalInput").ap()
    out = nc.dram_tensor("out", [128, NQ * NW], f16,
                         kind="ExternalOutput").ap()

    with tile.TileContext(nc) as tc:
        with (
            tc.tile_pool(name="persist", bufs=1) as persist,
            tc.tile_pool(name="band", bufs=10) as band_pool,
            tc.tile_pool(name="psum", bufs=4, space="PSUM") as psum_pool,
        ):
            f2_t = persist.tile([128, NQ, 2, 128], bf16, tag="f2")
            f1_t = persist.tile([128, FC, 2, FW], bf16, tag="f1")
            # 128KB h0-only copy of chunk 0's data (f2 pixels + f1 window)
            # as the very first transfer: the first matmul unblocks early
            # and the DVS grant onset tracks MM-start nearly 1:1
            # No early "head" transfer: an isolated first matmul followed
            # by a ~2us input gap keeps the HAM activity window at low
            # duty and DELAYS the full-speed grant (measured: grants land
            # ~3-7us after the stream becomes DENSE, not after the first
            # matmul).  Dense-from-the-first-pair wins.
            # zero the f1 col pads on-device (uniform across cores, unlike
            # the row pads); Vector is idle until its first cast
            nc.vector.memset(f1_t[:, 0:MD], 0)
            nc.vector.memset(f1_t[:, MD + W:], 0)
            # the 16 DMA engines round-robin descriptors across ALL
            # in-flight transfers regardless of queue, so every non-urgent
            # piece issued early dilutes the critical prefix's share of the
            # bus.  A dummy gpsimd memset (~1.3us) stream-delays the bulk
            # pieces so chunks 0-1's data gets ~70% of the bus instead of
            # ~35%, pulling the first matmul (and with it the whole dense
            # stream and the HAM grant) earlier.
            scratch = persist.tile([128, 1536], bf16, tag="scratch")
            nc.gpsimd.memset(scratch[:], 0)

            # inputs stream across BOTH HWDGE queues in parallel, in chunk-
            # consumption order (chunk q=2cc+rc needs f2 chunk q and f1
            # cols [8cc, 8cc+16)) — one queue alone delivers only
            # ~250 GB/s and would starve TensorE mid-stream.
            # the critical prefix (chunks 0-1 need f2[0:2] + f1[0:16]) is
            # split across BOTH queues so it lands at the combined rate:
            # the two HWDGE queues round-robin the shared DMA-engine pool,
            # so one queue alone gets only ~half the bus while both are
            # loaded.  Ranges are SBUF (padded) cols; DRAM is offset by -4.
            sync_pieces = [("f1", 4, 13), ("f1", 16, 24)]
            scal_pieces = [("f2", 0, 2), ("f1", 13, 16)]
            pool_pieces = [("f2", 2, 6), ("f1", 24, 40), ("f2", 6, 10),
                           ("f1", 40, 56), ("f2", 10, 16), ("f1", 56, 68)]
            for eng, pieces in ((nc.sync, sync_pieces),
                                (nc.scalar, scal_pieces),
                                (nc.gpsimd, pool_pieces)):
                for t, a, b in pieces:
                    if t == "f2":
                        eng.dma_start(out=f2_t[:, a:b], in_=f2d[:, a:b])
                    else:
                        eng.dma_start(out=f1_t[:, a:b],
                                      in_=f1d[:, a - MD:b - MD])

            for gi, (q0, n) in enumerate(GROUPS):
                # one PSUM bank per chunk (384 f32 in a 512-f32 slot)
                psum = psum_pool.tile([128, n, 512], f32)
                band = band_pool.tile([128, n, NW], f16)
                for qq in range(n):
                    q = q0 + qq
                    cc, rc = q // 2, q % 2
                    for h in range(2):
                        lhs = f2_t[:, q, h, :]
                        rhs = f1_t[:, CQ * cc:CQ * cc + CWIN, h,
                                   RQ * rc:RQ * rc + RWIN]
                        nc.tensor.matmul(
                            psum[:, qq, 0:NW], lhs, rhs,
                            start=(h == 0), stop=(h == 1),
                        )
                # casts alternate Vector/Scalar so cast throughput (the
                # psum-recycle pacer) is ~2x a single engine; Scalar's
                # ACT-table load is hoisted to its stream head where it
                # overlaps the input prefix harmlessly.
                if gi % 2 == 0:
                    nc.vector.tensor_copy(band[:, 0:n, :],
                                          psum[:, 0:n, 0:NW])
                else:
                    nc.scalar.copy(band[:, 0:n, :], psum[:, 0:n, 0:NW])
                # writes spread across three queues: gpsimd SWDGE takes
                # the Vector-cast groups' tail writes, Scalar the very
                # last (right after its own cast), Sync the rest — no
                # engine serializes two tail issues behind each other's
                # cast semaphores
                if gi in (2, 4, 6, 8):
                    eng = nc.gpsimd
                elif gi == 9:
                    eng = nc.scalar
                else:
                    eng = nc.sync
                eng.dma_start(out=out[:, q0 * NW:(q0 + n) * NW],
                              in_=band[:, 0:n, :])

    nc.compile()
    return nc


def _make_mask():
    """(128, 1, CWIN, RWIN) f32: band validity per partition (r_l, c_l)."""
    p = np.arange(128)
    rl = (p // CQ)[:, None, None]            # r' offset within chunk 0..15
    cl = (p % CQ)[:, None, None]             # c' offset within chunk 0..7
    cw = np.arange(CWIN)[None, :, None]
    rw = np.arange(RWIN)[None, None, :]
    m = ((rw >= rl) & (rw <= rl + 2 * MD)
         & (cw >= cl) & (cw <= cl + 2 * MD)).astype(np.float32)
    return m[:, None, :, :]


_MASK = _make_mask()


def _shard_inputs(feat1, feat2):
    """Per-core input dicts. Core i = (batch i//2, r'-half i%2)."""
    import ml_dtypes
    bf16 = ml_dtypes.bfloat16
    in_maps = []
    for i in range(N_CORES):
        b, rh = divmod(i, 2)
        r0 = rh * RSH
        # f2: (256, 32, 64) -> [128 chpart, 16 q=2cc+rc, 2 half, 128 pix]
        f2s = np.asarray(feat2[b, :, r0:r0 + RSH, :], dtype=bf16)
        f2i = np.ascontiguousarray(
            f2s.reshape(2, 128, NRC, RQ, NCC, CQ)
               .transpose(1, 4, 2, 0, 3, 5)       # chpart, cc, rc, h, rl, cl
               .reshape(128, NQ, 2, 128))
        # f1: rows [r0-4, r0+36) + cols +-4 zero-padded ->
        #     [128 chpart, 72 cpad, 2 half, 40 rpad]
        f1s = np.zeros((C, FW, W), bf16)
        lo = max(0, r0 - MD)
        hi = min(H, r0 + RSH + MD)
        f1s[:, lo - (r0 - MD):hi - (r0 - MD), :] = feat1[b, :, lo:hi, :]
        # [128 chpart, 64 real cols, 2 half, 40 rpad] (col pads added on
        # device via memset)
        f1i = np.ascontiguousarray(
            f1s.reshape(2, 128, FW, W).transpose(1, 3, 0, 2))
        in_maps.append({"f2": f2i, "f1": f1i})
    return in_maps


def _assemble(results):
    """Mask + scatter per-core compact bands into the dense output."""
    full = np.zeros((B, H, W, H, W), np.float32)
    for i in range(N_CORES):
        b, rh = divmod(i, 2)
        r0 = rh * RSH
        arr = (results[i]["out"].astype(np.float32)
               .reshape(128, NQ, CWIN, RWIN) * _MASK)
        arr = arr.reshape(RQ, CQ, NQ, CWIN, RWIN)
        # arr[rl, cl, q, cwin, rwin]; q = 2cc+rc;
        # r' = r0+16rc+rl, c' = 8cc+cl; r = r0+16rc+rwin-4, c = 8cc+cwin-4
        for q in range(NQ):
            cc, rc = divmod(q, 2)
            rbase = r0 + RQ * rc - MD
            cbase = CQ * cc - MD
            jlo, jhi = max(0, -rbase), min(RWIN, H - rbase)
            klo, khi = max(0, -cbase), min(CWIN, W - cbase)
            full[b, r0 + RQ * rc:r0 + RQ * rc + RQ, CQ * cc:CQ * cc + CQ,
                 rbase + jlo:rbase + jhi, cbase + klo:cbase + khi] = \
                arr[:, :, q, klo:khi, jlo:jhi].transpose(0, 1, 3, 2)
    return full.reshape(B, H * W, H, W)


def run(feat1, feat2, trace=False, trace_cores=None):
    """Returns (full output (B, H*W, H, W) float32, exec_time_ns or None)."""
    global _COMPILED
    from concourse.bass_utils import run_bass_kernel_spmd

    feat1 = np.asarray(feat1, dtype=np.float32)
    feat2 = np.asarray(feat2, dtype=np.float32)
    assert feat1.shape == (B, C, H, W) and feat2.shape == (B, C, H, W)

    if _COMPILED is None:
        _COMPILED = _build_program()
    nc = _COMPILED

    in_maps = _shard_inputs(feat1, feat2)
    res = run_bass_kernel_spmd(
        nc, in_maps, core_ids=list(range(N_CORES)),
        trace=trace, trace_cores=trace_cores,
    )
    return _assemble(res.results), res.exec_time_ns


def kernel(feat1, feat2):
    out, _ = run(feat1, feat2, trace=False)
    return out


# revision 27
# speedup vs baseline: 1.0976x; 1.0976x over previous
"""CostVolume kernel for Trainium2 (8 NeuronCores, Bass/Tile).

Math: the reference computes a 9x9-displacement correlation cost volume and
scatters it into out[b, r', c', r, c].  Substituting r' = r + di - 4,
c' = c + dj - 4 shows the output is a banded Gram matrix:

    out[b, r', c', r, c] = (sum_ch feat2[b,ch,r',c'] * feat1[b,ch,r,c])
                           * 1[|r'-r| <= 4] * 1[|c'-c| <= 4]

98% of the dense (B,H,W,H,W) output is structural zeros.  The device
computes and writes ONLY a compact band; the host applies the band mask
and scatters into a zero-initialized full-shape array during unsharding.

Sharding: 8 cores = 4 batches x 2 r'-halves (r' in [0,32) / [32,64)).

v2 layout (vs the 33us v1): chunks are 16x8 pixel tiles instead of 2x64.
A 128-partition chunk of (16 r' x 8 c') output pixels needs only the
f1 window (16+8) rows x (8+8) cols = 24x16 = 384 band columns, vs 640
for the old (2 r' x 64 c') shape.  384 f32 fits in ONE 2KiB PSUM bank,
so each (chunk, ch-half) is a single LDWEIGHTS+MATMUL pair: 32 pairs,
12288 streamed PE columns (vs 64 pairs / 20480 columns).  Cast work and
output bytes drop 1.67x as well (2.62 -> 1.57 MiB per core).

Per chunk q = 2*cc + rc (cc in 0..7 c'-blocks, rc in 0..1 r'-blocks):
  psum[p, cwin*24 + rwin] = sum_ch f2[ch, p] * f1[ch, 8cc+cwin, 16rc+rwin]
  with p = r_local*8 + c_local, f1 host-padded by 4 on all spatial edges.
bf16 matmul accumulated in fp32 PSUM, band cast to f16 on device, mask +
scatter on host.

Schedule (inherited from v1's perfetto/NTFF tuning):
- inputs stream across BOTH HWDGE queues (sync + scalar) in chunk-
  consumption order; a small h0-only head piece ("hd") lets the first
  matmul start ~1us before the first full f1/f2 pieces land.
- Vector casts groups 0-6, Scalar casts the last two groups (nothing
  follows them in its stream); ALL other out-DMA issues ride Sync, with
  two mid-stream writes on the otherwise-idle gpsimd SWDGE queue.
- group 0 and the last group are single-chunk: the first band write hits
  the output queue early, and the final cast+write tail is short.
"""

import numpy as np

B, C, H, W = 4, 256, 64, 64
MD = 4
N_CORES = 8
RSH = H // 2          # 32 r' rows per core
RQ, CQ = 16, 8        # chunk = 16 r' x 8 c' pixels
NRC, NCC = RSH // RQ, W // CQ   # 2 r'-blocks x 8 c'-blocks = 16 chunks
NQ = NRC * NCC
RWIN = RQ + 2 * MD    # 24 band rows per chunk
CWIN = CQ + 2 * MD    # 16 band cols per chunk
NW = RWIN * CWIN      # 384 band columns per chunk
FW = RSH + 2 * MD     # 40 padded f1 rows
FC = W + 2 * MD       # 72 padded f1 cols

# (group start chunk, chunks in group): doubles early (drain amortizes),
# singles for the last four chunks — small casts alternating across
# Vector/Scalar and small writes across three queues compress the tail
# between the last matmul and the NEFF teardown barrier
GROUPS = [(2 * g, 2) for g in range(6)] + [(q, 1) for q in range(12, 16)]

_COMPILED = None      # compiled Bacc program cache across kernel() calls


def _build_program():
    import concourse.bacc as bacc
    import concourse.tile as tile
    from concourse import mybir

    f32 = mybir.dt.float32
    f16 = mybir.dt.float16
    bf16 = mybir.dt.bfloat16

    nc = bacc.Bacc("TRN2", target_bir_lowering=False, debug=False,
                   num_devices=N_CORES)

    f2d = nc.dram_tensor("f2", [128, NQ, 2, 128], bf16,
                         kind="ExternalInput").ap()
    # f1 ships WITHOUT the 4-col zero pads on each side (64 real cols);
    # the pads are memset on-device, saving 160KB/core of HBM traffic
    f1d = nc.dram_tensor("f1", [128, W, 2, FW], bf16,
                         kind="ExternalInput").ap()
    out = nc.dram_tensor("out", [128, NQ * NW], f16,
                         kind="ExternalOutput").ap()

    with tile.TileContext(nc) as tc:
        with (
            tc.tile_pool(name="persist", bufs=1) as persist,
            tc.tile_pool(name="band", bufs=10) as band_pool,
            tc.tile_pool(name="psum", bufs=4, space="PSUM") as psum_pool,
        ):
            f2_t = persist.tile([128, NQ, 2, 128], bf16, tag="f2")
            f1_t = persist.tile([128, FC, 2, FW], bf16, tag="f1")
            # 128KB h0-only copy of chunk 0's data (f2 pixels + f1 window)
            # as the very first transfer: the first matmul unblocks early
            # and the DVS grant onset tracks MM-start nearly 1:1
            # No early "head" transfer: an isolated first matmul followed
            # by a ~2us input gap keeps the HAM activity window at low
            # duty and DELAYS the full-speed grant (measured: grants land
            # ~3-7us after the stream becomes DENSE, not after the first
            # matmul).  Dense-from-the-first-pair wins.
            # zero the f1 col pads on-device (uniform across cores, unlike
            # the row pads); Vector is idle until its first cast
            nc.vector.memset(f1_t[:, 0:MD], 0)
            nc.vector.memset(f1_t[:, MD + W:], 0)

            # inputs stream across BOTH HWDGE queues in parallel, in chunk-
            # consumption order (chunk q=2cc+rc needs f2 chunk q and f1
            # cols [8cc, 8cc+16)) — one queue alone delivers only
            # ~250 GB/s and would starve TensorE mid-stream.
            # the critical prefix (chunks 0-1 need f2[0:2] + f1[0:16]) is
            # split across BOTH queues so it lands at the combined rate:
            # the two HWDGE queues round-robin the shared DMA-engine pool,
            # so one queue alone gets only ~half the bus while both are
            # loaded.  Ranges are SBUF (padded) cols; DRAM is offset by -4.
            sync_pieces = [("f1", 4, 8), ("f1", 16, 24), ("f1", 24, 40),
                           ("f1", 40, 56), ("f1", 56, 68)]
            scal_pieces = [("f2", 0, 2), ("f1", 8, 16), ("f2", 2, 6),
                           ("f2", 6, 10), ("f2", 10, 16)]
            for eng, pieces in ((nc.sync, sync_pieces),
                                (nc.scalar, scal_pieces)):
                for t, a, b in pieces:
                    if t == "f2":
                        eng.dma_start(out=f2_t[:, a:b], in_=f2d[:, a:b])
                    else:
                        eng.dma_start(out=f1_t[:, a:b],
                                      in_=f1d[:, a - MD:b - MD])

            for gi, (q0, n) in enumerate(GROUPS):
                # one PSUM bank per chunk (384 f32 in a 512-f32 slot)
                psum = psum_pool.tile([128, n, 512], f32)
                band = band_pool.tile([128, n, NW], f16)
                for qq in range(n):
                    q = q0 + qq
                    cc, rc = q // 2, q % 2
                    for h in range(2):
                        lhs = f2_t[:, q, h, :]
                        rhs = f1_t[:, CQ * cc:CQ * cc + CWIN, h,
                                   RQ * rc:RQ * rc + RWIN]
                        nc.tensor.matmul(
                            psum[:, qq, 0:NW], lhs, rhs,
                            start=(h == 0), stop=(h == 1),
                        )
                # casts alternate Vector/Scalar so cast throughput (the
                # psum-recycle pacer) is ~2x a single engine; Scalar's
                # ACT-table load is hoisted to its stream head where it
                # overlaps the input prefix harmlessly.
                if gi % 2 == 0:
                    nc.vector.tensor_copy(band[:, 0:n, :],
                                          psum[:, 0:n, 0:NW])
                else:
                    nc.scalar.copy(band[:, 0:n, :], psum[:, 0:n, 0:NW])
                # writes spread across three queues: gpsimd SWDGE takes
                # the Vector-cast groups' tail writes, Scalar the very
                # last (right after its own cast), Sync the rest — no
                # engine serializes two tail issues behind each other's
                # cast semaphores
                if gi in (2, 4, 6, 8):
                    eng = nc.gpsimd
                elif gi == 9:
                    eng = nc.scalar
                else:
                    eng = nc.sync
                eng.dma_start(out=out[:, q0 * NW:(q0 + n) * NW],
                              in_=band[:, 0:n, :])

    nc.compile()
    return nc


def _make_mask():
    """(128, 1, CWIN, RWIN) f32: band validity per partition (r_l, c_l)."""
    p = np.arange(128)
    rl = (p // CQ)[:, None, None]            # r' offset within chunk 0..15
    cl = (p % CQ)[:, None, None]             # c' offset within chunk 0..7
    cw = np.arange(CWIN)[None, :, None]
    rw = np.arange(RWIN)[None, None, :]
    m = ((rw >= rl) & (rw <= rl + 2 * MD)
         & (cw >= cl) & (cw <= cl + 2 * MD)).astype(np.float32)
    return m[:, None, :, :]


_MASK = _make_mask()


def _shard_inputs(feat1, feat2):
    """Per-core input dicts. Core i = (batch i//2, r'-half i%2)."""
    import ml_dtypes
    bf16 = ml_dtypes.bfloat16
    in_maps = []
    for i in range(N_CORES):
        b, rh = divmod(i, 2)
        r0 = rh * RSH
        # f2: (256, 32, 64) -> [128 chpart, 16 q=2cc+rc, 2 half, 128 pix]
        f2s = np.asarray(feat2[b, :, r0:r0 + RSH, :], dtype=bf16)
        f2i = np.ascontiguousarray(
            f2s.reshape(2, 128, NRC, RQ, NCC, CQ)
               .transpose(1, 4, 2, 0, 3, 5)       # chpart, cc, rc, h, rl, cl
               .reshape(128, NQ, 2, 128))
        # f1: rows [r0-4, r0+36) + cols +-4 zero-padded ->
        #     [128 chpart, 72 cpad, 2 half, 40 rpad]
        f1s = np.zeros((C, FW, W), bf16)
        lo = max(0, r0 - MD)
        hi = min(H, r0 + RSH + MD)
        f1s[:, lo - (r0 - MD):hi - (r0 - MD), :] = feat1[b, :, lo:hi, :]
        # [128 chpart, 64 real cols, 2 half, 40 rpad] (col pads added on
        # device via memset)
        f1i = np.ascontiguousarray(
            f1s.reshape(2, 128, FW, W).transpose(1, 3, 0, 2))
        in_maps.append({"f2": f2i, "f1": f1i})
    return in_maps


def _assemble(results):
    """Mask + scatter per-core compact bands into the dense output."""
    full = np.zeros((B, H, W, H, W), np.float32)
    for i in range(N_CORES):
        b, rh = divmod(i, 2)
        r0 = rh * RSH
        arr = (results[i]["out"].astype(np.float32)
               .reshape(128, NQ, CWIN, RWIN) * _MASK)
        arr = arr.reshape(RQ, CQ, NQ, CWIN, RWIN)
        # arr[rl, cl, q, cwin, rwin]; q = 2cc+rc;
        # r' = r0+16rc+rl, c' = 8cc+cl; r = r0+16rc+rwin-4, c = 8cc+cwin-4
        for q in range(NQ):
            cc, rc = divmod(q, 2)
            rbase = r0 + RQ * rc - MD
            cbase = CQ * cc - MD
            jlo, jhi = max(0, -rbase), min(RWIN, H - rbase)
            klo, khi = max(0, -cbase), min(CWIN, W - cbase)
            full[b, r0 + RQ * rc:r0 + RQ * rc + RQ, CQ * cc:CQ * cc + CQ,
                 rbase + jlo:rbase + jhi, cbase + klo:cbase + khi] = \
                arr[:, :, q, klo:khi, jlo:jhi].transpose(0, 1, 3, 2)
    return full.reshape(B, H * W, H, W)


def run(feat1, feat2, trace=False, trace_cores=None):
    """Returns (full output (B, H*W, H, W) float32, exec_time_ns or None)."""
    global _COMPILED
    from concourse.bass_utils import run_bass_kernel_spmd

    feat1 = np.asarray(feat1, dtype=np.float32)
    feat2 = np.asarray(feat2, dtype=np.float32)
    assert feat1.shape == (B, C, H, W) and feat2.shape == (B, C, H, W)

    if _COMPILED is None:
        _COMPILED = _build_program()
    nc = _COMPILED

    in_maps = _shard_inputs(feat1, feat2)
    res = run_bass_kernel_spmd(
        nc, in_maps, core_ids=list(range(N_CORES)),
        trace=trace, trace_cores=trace_cores,
    )
    return _assemble(res.results), res.exec_time_ns


def kernel(feat1, feat2):
    out, _ = run(feat1, feat2, trace=False)
    return out
